# revision 1
# baseline (speedup 1.0000x reference)
"""Trainium2 kernel for nn_Attention_39204461478201.

The reference computes
    scores  = einsum('bqh,bkh->bqk', x, x) / sqrt(H)
    weights = softmax(scores, axis=1)          # over the q axis!
    context = einsum('bqk,bkh->bqh', weights, x)
    out     = mean(context, axis=1)
Because the softmax normalizes over axis=1 (q), every column of `weights`
sums to 1:  sum_q w[b,q,k] = 1 for all (b,k).  Therefore
    out[b,h] = (1/T) sum_q sum_k w[b,q,k] x[b,k,h]
             = (1/T) sum_k x[b,k,h] * (sum_q w[b,q,k])
             = mean(x, axis=1)[b,h]
— the attention collapses exactly to mean pooling over the time axis
(hence arch_category "pooling").

Device kernel: pure data parallel over 8 cores (2 batches/core).  Each
core streams its 8 MB slice from HBM and reduces it on the TensorEngine:
    psum[1,512] += w[128,1].T @ tile[128,512]     (PSUM-accumulated)
with w = 1/T = 2^-11.  Using float32r the PE streams 1 column/cycle
(~213 ns per [128,512] tile), so all compute hides under the DMA.

DMA layout (HW-tuned, see bench*.py):
  * rows grouped as "(p r)": partition p holds RB=16 *contiguous* rows,
    so every DMA is a fully linear HBM read (32 KB/partition chunks) —
    measured ~6% faster than the strided "(r p)" layout;
  * 1 MB DMAs alternating between the two physical HWDGE rings
    (sync + scalar sequencers) — saturates HBM at ~360 GB/s/core
    (23.3 us steady-state = the per-core HBM roofline);
  * batch 1's final DMAs shrink ([...,2,1,1] row-blocks) so the exposed
    tail after the last byte lands is just one matmul + PSUM copy + 2 KB
    output DMA.  Measured single-shot ~28.8 us/core (chained-NEFF method).
"""

import numpy as np

B, T, H = 16, 2048, 512
N_CORES = 8
B_PER = B // N_CORES     # batches per core
P = 128                  # SBUF partitions
RB = T // P              # 16 row-blocks of [128, H] per batch

# row-blocks per DMA; batch 0 hides under batch 1's stream, batch 1
# tapers so the last DMA is small (short exposed tail)
GROUPS = {0: [4, 4, 4, 4], 1: [4, 4, 4, 2, 1, 1]}

_prog_cache = {}


def _build_program(n_iters=1):
    if n_iters in _prog_cache:
        return _prog_cache[n_iters]

    import concourse.bass as bass
    import concourse.tile as tile
    from concourse import bacc, mybir

    nc = bacc.Bacc(
        "TRN2", target_bir_lowering=False, debug=False, num_devices=N_CORES
    )
    x = nc.dram_tensor("x", (B_PER, T, H), mybir.dt.float32r, kind="ExternalInput")
    out = nc.dram_tensor("out", (B_PER, H), mybir.dt.float32, kind="ExternalOutput")

    with tile.TileContext(nc) as tc:
        with (
            tc.tile_pool(name="w", bufs=1) as wpool,
            tc.tile_pool(name="xin", bufs=1) as xpool,
            tc.tile_pool(name="ps", bufs=B_PER, space=bass.MemorySpace.PSUM) as pspool,
            tc.tile_pool(name="res", bufs=B_PER) as respool,
        ):
            w = wpool.tile([P, 1], mybir.dt.float32)
            nc.vector.memset(w[:], 1.0 / T)
            w_r = w[:].bitcast(mybir.dt.float32r)
            seq = 0
            for _it in range(n_iters):
                for b in range(B_PER):
                    # partition p <- RB contiguous rows: fully linear DMA reads
                    xb = x.ap()[b].rearrange("(p r) h -> p r h", p=P)
                    ps = pspool.tile([1, H], mybir.dt.float32)
                    off = 0
                    n_done = 0
                    total = sum(GROUPS[b])
                    for i, g in enumerate(GROUPS[b]):
                        eng = nc.sync if seq % 2 == 0 else nc.scalar
                        seq += 1
                        t = xpool.tile([P, g, H], mybir.dt.float32r, tag=f"s{b}_{i}")
                        eng.dma_start(t[:], xb[:, off : off + g, :])
                        for r in range(g):
                            nc.tensor.matmul(
                                ps[:],
                                w_r,
                                t[:, r, :],
                                start=(n_done == 0),
                                stop=(n_done == total - 1),
                            )
                            n_done += 1
                        off += g
                    res = respool.tile([1, H], mybir.dt.float32)
                    nc.scalar.copy(res[:], ps[:])
                    nc.sync.dma_start(out.ap()[b : b + 1, :], res[:])
    nc.compile()
    _prog_cache[n_iters] = nc
    return nc


def kernel(lstm_out, **_unused):
    import os

    from concourse.bass_utils import run_bass_kernel_spmd

    x = np.ascontiguousarray(np.asarray(lstm_out), dtype=np.float32)
    assert x.shape == (B, T, H), x.shape
    in_maps = [{"x": x[i * B_PER : (i + 1) * B_PER]} for i in range(N_CORES)]
    nc = _build_program()
    core_ids = list(range(N_CORES))
    try:
        res = run_bass_kernel_spmd(nc, in_maps, core_ids=core_ids)
    except ModuleNotFoundError:
        # BASS_TRACE set but the axon NTFF hook isn't shipped in this
        # container (antenv.axon_hooks) — rerun with tracing disabled.
        os.environ["BASS_NEVER_TRACE"] = "1"
        res = run_bass_kernel_spmd(nc, in_maps, core_ids=core_ids)
    return np.concatenate([r["out"] for r in res.results], axis=0)

